# revision 1
# baseline (speedup 1.0000x reference)
"""Causal self-attention (B=2, T=2048, D=2048, 16 heads) on 8 trn2 cores.

Sharding: tensor-parallel over heads — 2 heads per core. Each core computes
q/k/v projections for its 2 heads (column-parallel), causal attention per
head, and a partial output projection (row-parallel). Host sums the 8
partial outputs.

Layout strategy per core (all matmuls contract over the partition dim):
  xT   [D_MODEL, B*T]   (host-pretransposed x)
  wqT  [D_MODEL, 256]   (Wq rows for this core's heads, transposed)
  qT_h [128, B*T]       = WqT_h.T @ xT   (head dim on partitions)
  S.T  [j, i] tiles     = kT_tile.T @ qT_chunk    (keys on partitions)
  PT   = exp(S.T / sqrt(128)) * causal_mask       (ACT, PSUM->SBUF)
  den  [1, i]           = ones.T @ PT   (PE partition-sum, accumulated)
  outT [d, i]           = v_tile.T @ PT (accumulated over j tiles)
  normalize: outT *= broadcast(1/den)   (GpSimd bcast + DVE mult)
  y    [t, m] partial   = outT_tile.T @ WoT_chunk (accum over 2 heads)
"""

import math
from contextlib import ExitStack

import numpy as np
import ml_dtypes

import concourse.bass as bass
import concourse.mybir as mybir
import concourse.tile as tile
from concourse import bacc
from concourse.bass_utils import run_bass_kernel_spmd
from concourse.masks import make_identity

P = 128
D_MODEL = 2048
NUM_HEADS = 16
D = 128            # head dim
B, T = 2, 2048
BT = B * T         # 4096
NCORES = 8
HPC = NUM_HEADS // NCORES   # 2 heads per core
KD = D_MODEL // P           # 16 d_model tiles
TJ = T // P                 # 16 key tiles per batch
IC = 512                    # query chunk width
NI = T // IC                # 4 query chunks per batch
TCH = BT // IC              # 8 token chunks for projections

F32 = mybir.dt.float32

_DT = {
    "f32": mybir.dt.float32,
    "f32r": mybir.dt.float32r,
    "bf16": mybir.dt.bfloat16,
}
_NP = {
    "f32": np.float32,
    "f32r": np.float32,
    "bf16": ml_dtypes.bfloat16,
}

F32R = mybir.dt.float32r


# dtype config: x/w = projection inputs, s = qT/kT storage (S matmul inputs),
# pt = exp'd probabilities, v = value tiles, o = outT storage (outproj lhsT),
# wo = Wo tiles. overlap = double-buffer qkv arrays across heads (more SBUF).
CFG_SAFE = dict(x="f32", w="f32", s="f32", pt="f32", v="f32", o="f32",
                wo="f32", overlap=False)
CFG_FAST = dict(x="bf16", w="bf16", s="bf16", pt="bf16", v="bf16", o="f32r",
                wo="f32r", overlap=True)
# validated: rel err 2.27e-4 vs fp32 reference, ~650 us on HW
CFG_F32R = dict(x="f32r", w="f32r", s="f32r", pt="f32r", v="f32r", o="f32r",
                wo="f32r", overlap=False)


def _emit(tc, cfg, xT, wqT, wkT, wvT, woT, y):
    nc = tc.nc
    x_dt = _DT[cfg["x"]]
    w_dt = _DT[cfg["w"]]
    s_dt = _DT[cfg["s"]]
    pt_dt = _DT[cfg["pt"]]
    v_dt = _DT[cfg["v"]]
    o_dt = _DT[cfg["o"]]
    wo_dt = _DT[cfg["wo"]]
    qb = 2 if cfg["overlap"] else 1   # bufs for per-head qkv arrays

    with ExitStack() as ctx:
        consts = ctx.enter_context(tc.tile_pool(name="consts", bufs=1))
        wpool = ctx.enter_context(tc.tile_pool(name="wpool", bufs=2))
        xpool = ctx.enter_context(tc.tile_pool(name="xpool", bufs=7))
        wopool = ctx.enter_context(tc.tile_pool(name="wopool", bufs=4))
        arrs = ctx.enter_context(tc.tile_pool(name="arrs", bufs=qb))
        arrs2 = ctx.enter_context(tc.tile_pool(name="arrs2", bufs=2))
        ptpool = ctx.enter_context(tc.tile_pool(name="ptpool", bufs=5))
        smalls = ctx.enter_context(tc.tile_pool(name="smalls", bufs=2))
        ypool = ctx.enter_context(tc.tile_pool(name="ypool", bufs=2))
        psum = ctx.enter_context(tc.tile_pool(name="psum", bufs=1, space="PSUM"))
        def _const(shape, dt, tag, fill_fn):
            # gpsimd memset/affine can't write f32r; stage in f32 then copy.
            if dt == F32R:
                stg = consts.tile([P, IC], F32, tag="stg",
                                  name="stg")[:shape[0], :shape[1]]
                fill_fn(stg)
                out = consts.tile(shape, dt, tag=tag, name=tag)
                nc.vector.tensor_copy(out, stg)
                return out
            out = consts.tile(shape, dt, tag=tag, name=tag)
            fill_fn(out)
            return out

        ident = _const([P, P], v_dt, "ident", lambda t: make_identity(nc, t))
        ones_col = _const([P, 1], pt_dt, "ones",
                          lambda t: nc.vector.memset(t, 1.0))

        # tri_mask[p, i] = 1.0 if i >= p else 0 (upper triangular keep)
        def _fill_tri(t):
            nc.gpsimd.memset(t, 0.0)
            nc.gpsimd.affine_select(
                out=t, in_=t, compare_op=mybir.AluOpType.is_gt,
                fill=1.0, base=0, pattern=[[-1, P]], channel_multiplier=1,
            )

        tri_mask = _const([P, P], pt_dt, "trimask", _fill_tri)

        xT3 = xT.rearrange("(ko p) t -> p ko t", p=P)
        w3 = {
            "q": wqT.rearrange("(ko p) o -> p ko o", p=P),
            "k": wkT.rearrange("(ko p) o -> p ko o", p=P),
            "v": wvT.rearrange("(ko p) o -> p ko o", p=P),
        }

        outTs = []
        scale = 1.0 / math.sqrt(D)

        for h in range(HPC):
            # ---- projections for head h: qT/kT [128, BT], vT -> v ----
            w_sb = {}
            for nm in ("q", "k", "v"):
                wt = wpool.tile([P, KD, D], w_dt, tag=f"w{nm}")
                nc.sync.dma_start(wt, w3[nm][:, :, h * D:(h + 1) * D])
                w_sb[nm] = wt
            qT = arrs.tile([P, BT], s_dt, tag="qT")
            kT = arrs.tile([P, BT], s_dt, tag="kT")
            vT = arrs.tile([P, BT], v_dt, tag="vT")
            dests = {"q": qT, "k": kT, "v": vT}
            for tch in range(TCH):
                tsl = slice(tch * IC, (tch + 1) * IC)
                ps = {nm: psum.tile([P, IC], F32, tag=f"s{i}", name=f"s{i}")
                      for i, nm in enumerate(("q", "k", "v"))}
                for kt in range(KD):
                    xt = xpool.tile([P, IC], x_dt, tag="xt")
                    nc.sync.dma_start(xt, xT3[:, kt, tsl])
                    for nm in ("q", "k", "v"):
                        nc.tensor.matmul(
                            ps[nm], w_sb[nm][:, kt], xt,
                            start=(kt == 0), stop=(kt == KD - 1),
                        )
                for nm in ("q", "k", "v"):
                    nc.vector.tensor_copy(dests[nm][:, tsl], ps[nm])

            # transpose vT -> v [128, B, TJ, D] (token tiles on partitions)
            v_sb = arrs.tile([P, B, TJ, D], v_dt, tag="v")
            for b in range(B):
                for jt in range(TJ):
                    pst = psum.tile([P, P], v_dt, tag="s3")
                    nc.tensor.transpose(
                        pst, vT[:, b * T + jt * P: b * T + (jt + 1) * P], ident)
                    nc.vector.tensor_copy(v_sb[:, b, jt], pst)

            # ---- attention for head h ----
            outT = arrs2.tile([P, BT], o_dt, tag="outT")
            outTs.append(outT)
            for b in range(B):
                for ic in range(NI):
                    isl = slice(b * T + ic * IC, b * T + (ic + 1) * IC)
                    nj = ic * 4 + 4          # causal: j tiles 0..nj-1
                    ck = h * B * NI + b * NI + ic
                    ps_o = psum.tile([P, IC], F32, tag=f"o{ck % 2}",
                                     name=f"o{ck % 2}")
                    pt_acc = smalls.tile([P, IC], pt_dt, tag="ptacc")
                    for jt in range(nj):
                        m = jt - ic * 4
                        # partial diagonal tiles: columns < m*128 are fully
                        # masked; restrict all work to the live sub-range.
                        lo = max(m, 0) * P
                        ps_s = psum.tile([P, IC], F32, tag=f"s{jt % 4}",
                                         name=f"s{jt % 4}")
                        nc.tensor.matmul(
                            ps_s[:, lo:],
                            kT[:, b * T + jt * P: b * T + (jt + 1) * P],
                            qT[:, b * T + ic * IC + lo:
                               b * T + (ic + 1) * IC], start=True, stop=True,
                        )
                        pt = ptpool.tile([P, IC], pt_dt, tag="pt")
                        nc.scalar.activation(
                            pt[:, lo:], ps_s[:, lo:],
                            mybir.ActivationFunctionType.Exp, scale=scale)
                        if m >= 0:
                            nc.vector.tensor_tensor(
                                pt[:, lo:lo + P], pt[:, lo:lo + P],
                                tri_mask, mybir.AluOpType.mult)
                        if jt == 0:
                            nc.vector.tensor_copy(pt_acc, pt)
                        else:
                            nc.vector.tensor_tensor(
                                pt_acc[:, lo:], pt_acc[:, lo:], pt[:, lo:],
                                mybir.AluOpType.add)
                        nc.tensor.matmul(
                            ps_o[:, lo:], v_sb[:, b, jt], pt[:, lo:],
                            start=(jt == 0), stop=(jt == nj - 1),
                            skip_group_check=True,
                        )
                    # denominators: one partition-sum matmul per chunk
                    ps_d = psum.tile([1, IC], F32, tag="den", name="den")
                    nc.tensor.matmul(ps_d, ones_col, pt_acc,
                                     start=True, stop=True,
                                     skip_group_check=True)
                    den_sb = smalls.tile([1, IC], F32, tag="densb")
                    nc.vector.tensor_copy(den_sb, ps_d)
                    bc = smalls.tile([P, IC], F32, tag="bc")
                    nc.gpsimd.partition_broadcast(bc, den_sb)
                    rb = smalls.tile([P, IC], F32, tag="rb")
                    nc.vector.reciprocal_approx_fast(out=rb, in_=bc)
                    nc.vector.tensor_tensor(
                        outT[:, isl], ps_o, rb, mybir.AluOpType.mult)

        # ---- output projection: y[t, m] partial over this core's heads ----
        woT3 = woT.rearrange("(h p) m -> h p m", p=P)
        for mc in range(D_MODEL // IC):
            msl = slice(mc * IC, (mc + 1) * IC)
            wo_sb = []
            for h in range(HPC):
                wt = wopool.tile([P, IC], wo_dt, tag="wo", name="wo")
                nc.sync.dma_start(wt, woT3[h, :, msl])
                wo_sb.append(wt)
            for tt in range(BT // P):
                ps_y = psum.tile([P, IC], F32, tag="y")
                for h in range(HPC):
                    nc.tensor.matmul(
                        ps_y, outTs[h][:, tt * P:(tt + 1) * P], wo_sb[h],
                        start=(h == 0), stop=(h == HPC - 1),
                    )
                y_sb = ypool.tile([P, IC], F32, tag="y")
                if mc == 0:
                    nc.vector.tensor_copy(y_sb, ps_y)
                else:
                    nc.scalar.copy(y_sb, ps_y)
                nc.sync.dma_start(y[tt * P:(tt + 1) * P, msl], y_sb)


def _build(cfg):
    nc = bacc.Bacc("TRN2", target_bir_lowering=False, debug=False,
                   num_devices=NCORES)
    xT = nc.dram_tensor("xT", [D_MODEL, BT], _DT[cfg["x"]],
                        kind="ExternalInput").ap()
    wqT = nc.dram_tensor("wqT", [D_MODEL, HPC * D], _DT[cfg["w"]],
                         kind="ExternalInput").ap()
    wkT = nc.dram_tensor("wkT", [D_MODEL, HPC * D], _DT[cfg["w"]],
                         kind="ExternalInput").ap()
    wvT = nc.dram_tensor("wvT", [D_MODEL, HPC * D], _DT[cfg["w"]],
                         kind="ExternalInput").ap()
    woT = nc.dram_tensor("woT", [HPC * D, D_MODEL], _DT[cfg["wo"]],
                         kind="ExternalInput").ap()
    y = nc.dram_tensor("y", [BT, D_MODEL], F32, kind="ExternalOutput").ap()
    with tile.TileContext(nc) as tc:
        _emit(tc, cfg, xT, wqT, wkT, wvT, woT, y)
    nc.compile()
    return nc


def _prep_inputs(x, Wq, Wk, Wv, Wo, cfg):
    xnp = _NP[cfg["x"]]
    wnp = _NP[cfg["w"]]
    wonp = _NP[cfg["wo"]]
    xT = np.ascontiguousarray(
        np.asarray(x, np.float32).reshape(BT, D_MODEL).T).astype(xnp)
    in_maps = []
    for c in range(NCORES):
        rows = slice(c * HPC * D, (c + 1) * HPC * D)
        in_maps.append({
            "xT": xT,
            "wqT": np.ascontiguousarray(np.asarray(Wq)[rows].T).astype(wnp),
            "wkT": np.ascontiguousarray(np.asarray(Wk)[rows].T).astype(wnp),
            "wvT": np.ascontiguousarray(np.asarray(Wv)[rows].T).astype(wnp),
            "woT": np.ascontiguousarray(
                np.asarray(Wo)[:, rows].T).astype(wonp),
        })
    return in_maps


def run(x, Wq, Wk, Wv, Wo, cfg=None, trace=False):
    cfg = cfg or CFG_F32R
    nc = _build(cfg)
    in_maps = _prep_inputs(x, Wq, Wk, Wv, Wo, cfg)
    try:
        res = run_bass_kernel_spmd(nc, in_maps, core_ids=list(range(NCORES)),
                                   trace=trace)
    except Exception:
        res = run_bass_kernel_spmd(nc, in_maps, core_ids=list(range(NCORES)),
                                   trace=trace)
    y = np.zeros((BT, D_MODEL), np.float32)
    for r in res.results:
        y += r["y"]
    return y.reshape(B, T, D_MODEL), res


def kernel(x, Wq, Wk, Wv, Wo):
    y, _ = run(x, Wq, Wk, Wv, Wo)
    return y



# revision 5
# speedup vs baseline: 1.8447x; 1.8447x over previous
"""Causal self-attention (B=2, T=2048, D=2048, 16 heads) on 8 trn2 cores.

Sharding: tensor-parallel over heads - 2 heads per core. Each core computes
q/k/v projections for its 2 heads (column-parallel), causal attention per
head, and a partial output projection (row-parallel). Host sums the 8
partial outputs.

v2 design (vs baseline): all-bf16 data path, single x pass with both heads
fused, v projected x-stationary directly into [token, d] layout (no PE
transposes), batch-pipelined emission so attention b0 overlaps proj b1 and
out-proj b0 overlaps attention b1, exact 8-bank PSUM layout, copies split
across DVE/ACT, few large DMAs.

Layouts (per core):
  xT4   [128, tch(8), kt(16), 512]   host-packed x, contiguous per DMA
  wq/wk [2h, 128(in-sub), kt, 128]   lhsT for W-stationary q/k projections
  wv    [2h, 128(in-sub), kt, 128]   rhs for x-stationary v projection
  wo    [2h, 128(d), 2048]           rhs (moving) for out-proj
  qT/kT [128(d), 2048] per (h, b)    head dim on partitions
  v     [128(tok), 16, 128] per (h,b) token tiles on partitions
  outT  [128(d), 2048] per (h, b)
  y     [4096, 2048]                 partial output (summed on host)
"""

import math
from contextlib import ExitStack

import numpy as np
import ml_dtypes

import concourse.bass as bass
import concourse.mybir as mybir
import concourse.tile as tile
from concourse import bacc
from concourse.bass_utils import run_bass_kernel_spmd

P = 128
DM = 2048          # d_model
B, T = 2, 2048
BT = B * T         # 4096
NCORES = 8
HPC = 2            # heads per core
D = 128            # head dim
KD = DM // P       # 16 contraction tiles
IC = 512           # query chunk width
NI = T // IC       # 4 query chunks per batch
TCH = BT // IC     # 8 token chunks
TJ = T // P        # 16 key tiles per batch

F32 = mybir.dt.float32
BF16 = mybir.dt.bfloat16
F32R = mybir.dt.float32r
_NP = {BF16: ml_dtypes.bfloat16, F32: np.float32, F32R: np.float32}

# s: qT/kT storage (S matmul inputs); o: outT storage (outproj lhsT);
# y: partial-output DMA dtype.
CFG_FAST = dict(s=BF16, o=BF16, y=BF16)
CFG_ACC = dict(s=F32R, o=F32R, y=F32)
DEBUG_DUMPS = False


def _emit(tc, cfg, xT4, wq, wk, wv, wo, y, dbg=None):
    nc = tc.nc
    s_dt = cfg["s"]
    o_dt = cfg["o"]
    scale = 1.0 / math.sqrt(D)

    with ExitStack() as ctx:
        consts = ctx.enter_context(tc.tile_pool(name="consts", bufs=1))
        wpool = ctx.enter_context(tc.tile_pool(name="wpool", bufs=1))
        xpool = ctx.enter_context(tc.tile_pool(name="xpool", bufs=2))
        arrs = ctx.enter_context(tc.tile_pool(name="arrs", bufs=1))
        ptpool = ctx.enter_context(tc.tile_pool(name="ptpool", bufs=4))
        accpool = ctx.enter_context(tc.tile_pool(name="accpool", bufs=2))
        smalls = ctx.enter_context(tc.tile_pool(name="smalls", bufs=2))
        ypool = ctx.enter_context(tc.tile_pool(name="ypool", bufs=2))
        psum = ctx.enter_context(tc.tile_pool(name="psum", bufs=1, space="PSUM"))

        # ---- constants ----
        ones_col = consts.tile([P, 1], BF16, tag="ones", name="ones")
        nc.vector.memset(ones_col, 1.0)

        # tri[p, i] = 1.0 if i >= p else 0 (keep lower-triangular in S.T)
        tri = consts.tile([P, P], BF16, tag="tri", name="tri")
        nc.gpsimd.memset(tri, 0.0)
        nc.gpsimd.affine_select(
            out=tri, in_=tri, compare_op=mybir.AluOpType.is_gt,
            fill=1.0, base=0, pattern=[[-1, P]], channel_multiplier=1,
        )

        # ---- persistent weights ----
        wq_sb, wk_sb, wv_sb, wo_sb = [], [], [], []
        for h in range(HPC):
            for lst, src, nm in ((wq_sb, wq, "wq"), (wk_sb, wk, "wk"),
                                 (wv_sb, wv, "wv")):
                t = wpool.tile([P, KD, P], BF16, tag=f"{nm}{h}", name=f"{nm}{h}")
                nc.sync.dma_start(t, src[h])
                lst.append(t)
            t = wpool.tile([P, DM], BF16, tag=f"wo{h}", name=f"wo{h}")
            nc.sync.dma_start(t, wo[h])
            wo_sb.append(t)

        # ---- per-(head, batch) arrays ----
        qT = [[arrs.tile([P, T], s_dt, tag=f"qT{h}{b}", name=f"qT{h}{b}")
               for b in range(B)] for h in range(HPC)]
        kT = [[arrs.tile([P, T], s_dt, tag=f"kT{h}{b}", name=f"kT{h}{b}")
               for b in range(B)] for h in range(HPC)]
        v_sb = [[arrs.tile([P, TJ, D], BF16, tag=f"v{h}{b}", name=f"v{h}{b}")
                 for b in range(B)] for h in range(HPC)]
        outT = [[arrs.tile([P, T], o_dt, tag=f"oT{h}{b}", name=f"oT{h}{b}")
                 for b in range(B)] for h in range(HPC)]

        eng = [0]  # alternating copy-engine counter

        def copy_out(dst, src):
            if eng[0] % 2 == 0:
                nc.vector.tensor_copy(dst, src)
            else:
                nc.scalar.copy(dst, src)
            eng[0] += 1

        # ---- QKV projection for one 512-token chunk (both heads) ----
        def proj_tch(tch):
            b, tc4 = tch // 4, tch % 4
            tsl = slice(tc4 * IC, (tc4 + 1) * IC)
            xt = xpool.tile([P, KD, IC], BF16, tag="xt", name="xt")
            nc.sync.dma_start(xt, xT4[:, tch])
            for h in range(HPC):
                psq = psum.tile([P, IC], F32, tag="pq", name="pq")
                psk = psum.tile([P, IC], F32, tag="pk", name="pk")
                psv = psum.tile([P, IC], F32, tag="pv", name="pv")
                for kt in range(KD):
                    st, sp = kt == 0, kt == KD - 1
                    nc.tensor.matmul(psq, wq_sb[h][:, kt], xt[:, kt],
                                     start=st, stop=sp)
                    nc.tensor.matmul(psk, wk_sb[h][:, kt], xt[:, kt],
                                     start=st, stop=sp)
                    for sub in range(4):
                        # start=True clears has_written for the whole bank, so
                        # only the bank's first matmul may set it; the other
                        # kt=0 sub-tiles write via their cleared bits.
                        nc.tensor.matmul(
                            psv[:, sub * D:(sub + 1) * D],
                            xt[:, kt, sub * P:(sub + 1) * P], wv_sb[h][:, kt],
                            start=(st and sub == 0), stop=(sp and sub == 3),
                            skip_group_check=True)
                copy_out(qT[h][b][:, tsl], psq)
                copy_out(kT[h][b][:, tsl], psk)
                copy_out(v_sb[h][b][:, tc4 * 4:(tc4 + 1) * 4], psv)

        # ---- attention for one (head, batch, 512-query chunk) ----
        def attn_chunk(h, b, ic):
            nj = 4 * (ic + 1)
            qoff = ic * IC
            ck = (h * B + b) * NI + ic
            ps_o = psum.tile([P, IC], F32, tag=f"o{ck % 2}", name=f"o{ck % 2}")
            pt_acc = accpool.tile([P, IC], BF16, tag="ptacc", name="ptacc")
            for jt in range(nj):
                m = jt - ic * 4
                lo = max(m, 0) * P
                ps_s = psum.tile([P, IC], F32, tag=f"s{jt % 2}",
                                 name=f"s{jt % 2}")
                nc.tensor.matmul(
                    ps_s[:, lo:], kT[h][b][:, jt * P:(jt + 1) * P],
                    qT[h][b][:, qoff + lo:qoff + IC], start=True, stop=True)
                pt = ptpool.tile([P, IC], BF16, tag="pt", name="pt")
                nc.scalar.activation(
                    pt[:, lo:], ps_s[:, lo:],
                    mybir.ActivationFunctionType.Exp, scale=scale)
                if m >= 0:
                    nc.vector.tensor_tensor(
                        pt[:, lo:lo + P], pt[:, lo:lo + P], tri,
                        mybir.AluOpType.mult)
                if jt == 0:
                    nc.vector.tensor_copy(pt_acc, pt)
                else:
                    nc.vector.tensor_tensor(
                        pt_acc[:, lo:], pt_acc[:, lo:], pt[:, lo:],
                        mybir.AluOpType.add)
                nc.tensor.matmul(
                    ps_o[:, lo:], v_sb[h][b][:, jt], pt[:, lo:],
                    start=(jt == 0), stop=(jt == nj - 1),
                    skip_group_check=True)
            # denominator: one partition-sum matmul on the accumulated probs
            ps_d = psum.tile([1, IC], F32, tag="den", name="den")
            nc.tensor.matmul(ps_d, ones_col, pt_acc, start=True, stop=True,
                             skip_group_check=True)
            den_sb = smalls.tile([1, IC], F32, tag="densb", name="densb")
            nc.scalar.copy(den_sb, ps_d)
            rb = smalls.tile([1, IC], F32, tag="rb", name="rb")
            nc.vector.reciprocal_approx_fast(out=rb, in_=den_sb)
            bc = smalls.tile([P, IC], F32, tag="bc", name="bc")
            nc.gpsimd.partition_broadcast(bc, rb)
            nc.vector.tensor_tensor(
                outT[h][b][:, qoff:qoff + IC], ps_o, bc, mybir.AluOpType.mult)

        # ---- out projection for one 128-token tile ----
        def outproj_tt(tt):
            b, jt = tt // TJ, tt % TJ
            y_sb = ypool.tile([P, DM], cfg["y"], tag="ysb", name="ysb")
            for mc in range(4):
                msl = slice(mc * IC, (mc + 1) * IC)
                psy = psum.tile([P, IC], F32, tag=("pq", "pk", "pv")[mc % 3],
                                name="py")
                for h in range(HPC):
                    nc.tensor.matmul(
                        psy, outT[h][b][:, jt * P:(jt + 1) * P],
                        wo_sb[h][:, msl], start=(h == 0), stop=(h == HPC - 1))
                copy_out(y_sb[:, msl], psy)
            nc.sync.dma_start(y[tt * P:(tt + 1) * P, :], y_sb)

        # ---- emission: batch-pipelined ----
        for tch in range(4):                      # proj b0
            proj_tch(tch)
        for i, tch in enumerate(range(4, 8)):     # proj b1 || attn b0
            proj_tch(tch)
            attn_chunk(0, 0, i)
            attn_chunk(1, 0, i)
        for ic in range(NI):                      # attn b1 || outproj b0
            attn_chunk(0, 1, ic)
            attn_chunk(1, 1, ic)
            for tt in range(ic * 4, ic * 4 + 4):
                outproj_tt(tt)
        for tt in range(TJ, 2 * TJ):              # outproj b1
            outproj_tt(tt)

        if dbg is not None:
            for h in range(HPC):
                for b in range(B):
                    nc.sync.dma_start(dbg["qT"][h, b], qT[h][b])
                    nc.sync.dma_start(dbg["kT"][h, b], kT[h][b])
                    nc.sync.dma_start(dbg["v"][h, b], v_sb[h][b])
                    nc.sync.dma_start(dbg["outT"][h, b], outT[h][b])


def _build(cfg):
    nc = bacc.Bacc("TRN2", target_bir_lowering=False, debug=False,
                   num_devices=NCORES)
    xT4 = nc.dram_tensor("xT4", [P, TCH, KD, IC], BF16,
                         kind="ExternalInput").ap()
    wq = nc.dram_tensor("wq", [HPC, P, KD, P], BF16, kind="ExternalInput").ap()
    wk = nc.dram_tensor("wk", [HPC, P, KD, P], BF16, kind="ExternalInput").ap()
    wv = nc.dram_tensor("wv", [HPC, P, KD, P], BF16, kind="ExternalInput").ap()
    wo = nc.dram_tensor("wo", [HPC, P, DM], BF16, kind="ExternalInput").ap()
    y = nc.dram_tensor("y", [BT, DM], cfg["y"], kind="ExternalOutput").ap()
    dbg = None
    if DEBUG_DUMPS:
        dbg = {
            "qT": nc.dram_tensor("dqT", [HPC, B, P, T], cfg["s"],
                                 kind="ExternalOutput").ap(),
            "kT": nc.dram_tensor("dkT", [HPC, B, P, T], cfg["s"],
                                 kind="ExternalOutput").ap(),
            "v": nc.dram_tensor("dv", [HPC, B, P, TJ, D], BF16,
                                kind="ExternalOutput").ap(),
            "outT": nc.dram_tensor("doutT", [HPC, B, P, T], cfg["o"],
                                   kind="ExternalOutput").ap(),
        }
    with tile.TileContext(nc) as tc:
        _emit(tc, cfg, xT4, wq, wk, wv, wo, y, dbg)
    nc.compile()
    return nc


def _prep_inputs(x, Wq, Wk, Wv, Wo):
    bf = ml_dtypes.bfloat16
    xt = np.asarray(x, np.float32).reshape(BT, DM)
    # [p, tch, kt, 512], contiguous per (p, tch)
    xT4 = np.ascontiguousarray(
        xt.reshape(TCH, IC, KD, P).transpose(3, 0, 2, 1)).astype(bf)

    def wqkv(W, c):
        Wc = np.asarray(W, np.float32)[c * HPC * D:(c + 1) * HPC * D]
        return np.ascontiguousarray(
            Wc.reshape(HPC, P, KD, P).transpose(0, 3, 2, 1)).astype(bf)

    in_maps = []
    for c in range(NCORES):
        Woc = np.asarray(Wo, np.float32)[:, c * HPC * D:(c + 1) * HPC * D]
        in_maps.append({
            "xT4": xT4,
            "wq": wqkv(Wq, c),
            "wk": wqkv(Wk, c),
            "wv": wqkv(Wv, c),
            "wo": np.ascontiguousarray(
                Woc.reshape(DM, HPC, P).transpose(1, 2, 0)).astype(bf),
        })
    return in_maps


def run(x, Wq, Wk, Wv, Wo, cfg=None, trace=False):
    cfg = cfg or CFG_FAST
    nc = _build(cfg)
    in_maps = _prep_inputs(x, Wq, Wk, Wv, Wo)
    try:
        res = run_bass_kernel_spmd(nc, in_maps, core_ids=list(range(NCORES)),
                                   trace=trace)
    except Exception:
        res = run_bass_kernel_spmd(nc, in_maps, core_ids=list(range(NCORES)),
                                   trace=trace)
    y = np.zeros((BT, DM), np.float32)
    for r in res.results:
        y += np.asarray(r["y"], np.float32)
    return y.reshape(B, T, DM), res


def kernel(x, Wq, Wk, Wv, Wo):
    y, _ = run(x, Wq, Wk, Wv, Wo)
    return y


# revision 15
# speedup vs baseline: 1.9254x; 1.0437x over previous
"""Causal self-attention (B=2, T=2048, D=2048, 16 heads) on 8 trn2 cores.

Sharding: tensor-parallel over heads - 2 heads per core. Each core computes
q/k/v projections for its 2 heads (column-parallel), causal attention per
head, and a partial output projection (row-parallel). Host sums the 8
partial outputs.

v2 design (vs baseline): all-bf16 data path, single x pass with both heads
fused, v projected x-stationary directly into [token, d] layout (no PE
transposes), batch-pipelined emission so attention b0 overlaps proj b1 and
out-proj b0 overlaps attention b1, exact 8-bank PSUM layout, copies split
across DVE/ACT, few large DMAs.

Layouts (per core):
  xT4   [128, tch(8), kt(16), 512]   host-packed x, contiguous per DMA
  wq/wk [2h, 128(in-sub), kt, 128]   lhsT for W-stationary q/k projections
  wv    [2h, 128(in-sub), kt, 128]   rhs for x-stationary v projection
  wo    [2h, 128(d), 2048]           rhs (moving) for out-proj
  qT/kT [128(d), 2048] per (h, b)    head dim on partitions
  v     [128(tok), 16, 128] per (h,b) token tiles on partitions
  outT  [128(d), 2048] per (h, b)
  y     [4096, 2048]                 partial output (summed on host)
"""

import math
from contextlib import ExitStack

import numpy as np
import ml_dtypes

import concourse.bass as bass
import concourse.mybir as mybir
import concourse.tile as tile
from concourse import bacc
from concourse.bass_utils import run_bass_kernel_spmd

P = 128
DM = 2048          # d_model
B, T = 2, 2048
BT = B * T         # 4096
NCORES = 8
HPC = 2            # heads per core
D = 128            # head dim
KD = DM // P       # 16 contraction tiles
IC = 512           # query chunk width
NI = T // IC       # 4 query chunks per batch
TCH = BT // IC     # 8 token chunks
TJ = T // P        # 16 key tiles per batch

F32 = mybir.dt.float32
BF16 = mybir.dt.bfloat16
F32R = mybir.dt.float32r
_NP = {BF16: ml_dtypes.bfloat16, F32: np.float32, F32R: np.float32}

# s: qT/kT storage (S matmul inputs); o: outT storage (outproj lhsT);
# y: partial-output DMA dtype.
CFG_FAST = dict(s=BF16, o=BF16, y=BF16)
CFG_ACC = dict(s=F32R, o=F32R, y=F32)
DEBUG_DUMPS = False


def _emit(tc, cfg, xT4, wq, wk, wv, wo, y, dbg=None):
    nc = tc.nc
    s_dt = cfg["s"]
    o_dt = cfg["o"]
    scale = 1.0 / math.sqrt(D)

    with ExitStack() as ctx:
        consts = ctx.enter_context(tc.tile_pool(name="consts", bufs=1))
        wpool = ctx.enter_context(tc.tile_pool(name="wpool", bufs=1))
        xpool = ctx.enter_context(tc.tile_pool(name="xpool", bufs=2))
        arrs = ctx.enter_context(tc.tile_pool(name="arrs", bufs=1))
        ptpool = ctx.enter_context(tc.tile_pool(name="ptpool", bufs=4))
        accpool = ctx.enter_context(tc.tile_pool(name="accpool", bufs=2))
        smalls = ctx.enter_context(tc.tile_pool(name="smalls", bufs=2))
        ypool = ctx.enter_context(tc.tile_pool(name="ypool", bufs=2))
        psum = ctx.enter_context(tc.tile_pool(name="psum", bufs=1, space="PSUM"))

        # ---- constants ----
        ones_col = consts.tile([P, 1], BF16, tag="ones", name="ones")
        nc.vector.memset(ones_col, 1.0)

        # tri[p, i] = 1.0 if i >= p else 0 (keep lower-triangular in S.T)
        tri = consts.tile([P, P], BF16, tag="tri", name="tri")
        nc.gpsimd.memset(tri, 0.0)
        nc.gpsimd.affine_select(
            out=tri, in_=tri, compare_op=mybir.AluOpType.is_gt,
            fill=1.0, base=0, pattern=[[-1, P]], channel_multiplier=1,
        )

        # ---- warmup: dense dummy matmuls warm the PE clock (HAM) while the
        # first DMAs are in flight; they retire before real work is ready.
        warm = consts.tile([P, IC], BF16, tag="warm", name="warm")
        nc.vector.memset(warm, 0.0)
        ps_w = psum.tile([P, IC], F32, tag="o0", name="warmps")
        for _ in range(28):
            nc.tensor.matmul(ps_w, warm[:, :P], warm, start=True, stop=True,
                             skip_group_check=True)

        # ---- persistent weights (tiles now; DMAs issued in emission order
        # below so x/h0 weights land first) ----
        wq_sb, wk_sb, wv_sb, wo_sb = [], [], [], []
        for h in range(HPC):
            for lst, nm in ((wq_sb, "wq"), (wk_sb, "wk"), (wv_sb, "wv")):
                lst.append(wpool.tile([P, KD, P], BF16, tag=f"{nm}{h}",
                                      name=f"{nm}{h}"))
            wo_sb.append(wpool.tile([P, DM], BF16, tag=f"wo{h}",
                                    name=f"wo{h}"))

        def load_w(h):
            for t, src in ((wq_sb[h], wq), (wk_sb[h], wk), (wv_sb[h], wv)):
                nc.sync.dma_start(t, src[h])

        # ---- per-(head, batch) arrays ----
        qT = [[arrs.tile([P, T], s_dt, tag=f"qT{h}{b}", name=f"qT{h}{b}")
               for b in range(B)] for h in range(HPC)]
        kT = [[arrs.tile([P, T], s_dt, tag=f"kT{h}{b}", name=f"kT{h}{b}")
               for b in range(B)] for h in range(HPC)]
        v_sb = [[arrs.tile([P, TJ, D], BF16, tag=f"v{h}{b}", name=f"v{h}{b}")
                 for b in range(B)] for h in range(HPC)]
        outT = [[arrs.tile([P, T], o_dt, tag=f"oT{h}{b}", name=f"oT{h}{b}")
                 for b in range(B)] for h in range(HPC)]

        eng = [0]  # alternating copy-engine counter

        def copy_out(dst, src):
            if eng[0] % 2 == 0:
                nc.vector.tensor_copy(dst, src)
            else:
                nc.scalar.copy(dst, src)
            eng[0] += 1

        # ---- QKV projection for one 512-token chunk (both heads) ----
        def proj_tch(tch, xt=None):
            b, tc4 = tch // 4, tch % 4
            tsl = slice(tc4 * IC, (tc4 + 1) * IC)
            if xt is None:
                xt = xpool.tile([P, KD, IC], BF16, tag="xt", name="xt")
                nc.sync.dma_start(xt, xT4[:, tch])
            for h in range(HPC):
                psq = psum.tile([P, IC], F32, tag="pq", name="pq")
                psk = psum.tile([P, IC], F32, tag="pk", name="pk")
                psv = psum.tile([P, IC], F32, tag="pv", name="pv")
                for kt in range(KD):
                    st, sp = kt == 0, kt == KD - 1
                    nc.tensor.matmul(psq, wq_sb[h][:, kt], xt[:, kt],
                                     start=st, stop=sp)
                    nc.tensor.matmul(psk, wk_sb[h][:, kt], xt[:, kt],
                                     start=st, stop=sp)
                    for sub in range(4):
                        # start=True clears has_written for the whole bank, so
                        # only the bank's first matmul may set it; the other
                        # kt=0 sub-tiles write via their cleared bits.
                        nc.tensor.matmul(
                            psv[:, sub * D:(sub + 1) * D],
                            xt[:, kt, sub * P:(sub + 1) * P], wv_sb[h][:, kt],
                            start=(st and sub == 0), stop=(sp and sub == 3),
                            skip_group_check=True)
                copy_out(qT[h][b][:, tsl], psq)
                copy_out(kT[h][b][:, tsl], psk)
                copy_out(v_sb[h][b][:, tc4 * 4:(tc4 + 1) * 4], psv)

        # ---- attention for one (head, batch, 512-query chunk) ----
        cseq = [0]  # emission-order chunk counter for psum rotation

        def attn_chunk(h, b, ic):
            nj = 4 * (ic + 1)
            qoff = ic * IC
            ck = cseq[0]
            cseq[0] += 1
            qs = qT[h][b][:, qoff:qoff + IC]
            ps_o = psum.tile([P, IC], F32, tag=f"o{ck % 2}", name=f"o{ck % 2}")
            pt_acc = accpool.tile([P, IC], BF16, tag="ptacc", name="ptacc")
            for jt in range(nj):
                m = jt - ic * 4
                lo = max(m, 0) * P
                ps_s = psum.tile([P, IC], F32, tag=f"s{jt % 2}",
                                 name=f"s{jt % 2}")
                nc.tensor.matmul(
                    ps_s[:, lo:], kT[h][b][:, jt * P:(jt + 1) * P],
                    qs[:, lo:], start=True, stop=True)
                pt = ptpool.tile([P, IC], BF16, tag="pt", name="pt")
                nc.scalar.activation(
                    pt[:, lo:], ps_s[:, lo:],
                    mybir.ActivationFunctionType.Exp, scale=scale)
                if m >= 0:
                    nc.vector.tensor_tensor(
                        pt[:, lo:lo + P], pt[:, lo:lo + P], tri,
                        mybir.AluOpType.mult)
                if jt == 0:
                    nc.vector.tensor_copy(pt_acc, pt)
                else:
                    nc.vector.tensor_tensor(
                        pt_acc[:, lo:], pt_acc[:, lo:], pt[:, lo:],
                        mybir.AluOpType.add)
                nc.tensor.matmul(
                    ps_o[:, lo:], v_sb[h][b][:, jt], pt[:, lo:],
                    start=(jt == 0), stop=(jt == nj - 1),
                    skip_group_check=True)
            # denominator: one partition-sum matmul on the accumulated probs
            ps_d = psum.tile([1, IC], F32, tag="den", name="den")
            nc.tensor.matmul(ps_d, ones_col, pt_acc, start=True, stop=True,
                             skip_group_check=True)
            den_sb = smalls.tile([1, IC], F32, tag="densb", name="densb")
            nc.scalar.copy(den_sb, ps_d)
            rb = smalls.tile([1, IC], F32, tag="rb", name="rb")
            nc.vector.reciprocal_approx_fast(out=rb, in_=den_sb)
            bc = smalls.tile([P, IC], F32, tag="bc", name="bc")
            nc.gpsimd.partition_broadcast(bc, rb)
            nc.vector.tensor_tensor(
                outT[h][b][:, qoff:qoff + IC], ps_o, bc, mybir.AluOpType.mult)

        # ---- out projection for one 128-token tile ----
        def outproj_tt(tt):
            b, jt = tt // TJ, tt % TJ
            y_sb = ypool.tile([P, DM], cfg["y"], tag="ysb", name="ysb")
            for mc in range(4):
                msl = slice(mc * IC, (mc + 1) * IC)
                psy = psum.tile([P, IC], F32, tag=("pq", "pk", "pv")[mc % 3],
                                name="py")
                for h in range(HPC):
                    nc.tensor.matmul(
                        psy, outT[h][b][:, jt * P:(jt + 1) * P],
                        wo_sb[h][:, msl], start=(h == 0), stop=(h == HPC - 1))
                copy_out(y_sb[:, msl], psy)
            nc.sync.dma_start(y[tt * P:(tt + 1) * P, :], y_sb)

        # ---- emission: batch-pipelined ----
        xt0 = xpool.tile([P, KD, IC], BF16, tag="xt", name="xt")
        nc.sync.dma_start(xt0, xT4[:, 0])         # x first, then h0 weights
        load_w(0)
        load_w(1)
        nc.sync.dma_start(wo_sb[0], wo[0])
        nc.sync.dma_start(wo_sb[1], wo[1])
        proj_tch(0, xt=xt0)
        for tch in range(1, 4):                   # proj b0
            proj_tch(tch)
        attn_chunk(0, 0, 0)                       # proj b1 || attn b0
        attn_chunk(1, 0, 0)
        proj_tch(4)
        attn_chunk(0, 0, 1)
        attn_chunk(1, 0, 1)
        proj_tch(5)
        attn_chunk(0, 0, 2)
        attn_chunk(1, 0, 2)
        proj_tch(6)
        attn_chunk(0, 0, 3)
        proj_tch(7)
        attn_chunk(1, 0, 3)
        for ic in range(NI):                      # attn b1 || outproj b0
            attn_chunk(0, 1, ic)
            attn_chunk(1, 1, ic)
            for tt in range(ic * 4, ic * 4 + 4):
                outproj_tt(tt)
        for tt in range(TJ, 2 * TJ):              # outproj b1
            outproj_tt(tt)

        if dbg is not None:
            for h in range(HPC):
                nc.sync.dma_start(dbg["wo"][h], wo_sb[h])
                for b in range(B):
                    nc.sync.dma_start(dbg["qT"][h, b], qT[h][b])
                    nc.sync.dma_start(dbg["kT"][h, b], kT[h][b])
                    nc.sync.dma_start(dbg["v"][h, b], v_sb[h][b])
                    nc.sync.dma_start(dbg["outT"][h, b], outT[h][b])


def _build(cfg):
    nc = bacc.Bacc("TRN2", target_bir_lowering=False, debug=False,
                   num_devices=NCORES)
    xT4 = nc.dram_tensor("xT4", [P, TCH, KD, IC], BF16,
                         kind="ExternalInput").ap()
    wq = nc.dram_tensor("wq", [HPC, P, KD, P], BF16, kind="ExternalInput").ap()
    wk = nc.dram_tensor("wk", [HPC, P, KD, P], BF16, kind="ExternalInput").ap()
    wv = nc.dram_tensor("wv", [HPC, P, KD, P], BF16, kind="ExternalInput").ap()
    wo = nc.dram_tensor("wo", [HPC, P, DM], BF16, kind="ExternalInput").ap()
    y = nc.dram_tensor("y", [BT, DM], cfg["y"], kind="ExternalOutput").ap()
    dbg = None
    if DEBUG_DUMPS:
        dbg = {
            "qT": nc.dram_tensor("dqT", [HPC, B, P, T], cfg["s"],
                                 kind="ExternalOutput").ap(),
            "kT": nc.dram_tensor("dkT", [HPC, B, P, T], cfg["s"],
                                 kind="ExternalOutput").ap(),
            "v": nc.dram_tensor("dv", [HPC, B, P, TJ, D], BF16,
                                kind="ExternalOutput").ap(),
            "outT": nc.dram_tensor("doutT", [HPC, B, P, T], cfg["o"],
                                   kind="ExternalOutput").ap(),
            "wo": nc.dram_tensor("dwo", [HPC, P, DM], BF16,
                                 kind="ExternalOutput").ap(),
        }
    with tile.TileContext(nc) as tc:
        _emit(tc, cfg, xT4, wq, wk, wv, wo, y, dbg)
    nc.compile()
    return nc


def _prep_inputs(x, Wq, Wk, Wv, Wo):
    bf = ml_dtypes.bfloat16
    xt = np.asarray(x, np.float32).reshape(BT, DM)
    # [p, tch, kt, 512], contiguous per (p, tch)
    xT4 = np.ascontiguousarray(
        xt.reshape(TCH, IC, KD, P).transpose(3, 0, 2, 1)).astype(bf)

    def wqkv(W, c):
        Wc = np.asarray(W, np.float32)[c * HPC * D:(c + 1) * HPC * D]
        return np.ascontiguousarray(
            Wc.reshape(HPC, P, KD, P).transpose(0, 3, 2, 1)).astype(bf)

    in_maps = []
    for c in range(NCORES):
        Woc = np.asarray(Wo, np.float32)[:, c * HPC * D:(c + 1) * HPC * D]
        in_maps.append({
            "xT4": xT4,
            "wq": wqkv(Wq, c),
            "wk": wqkv(Wk, c),
            "wv": wqkv(Wv, c),
            "wo": np.ascontiguousarray(
                Woc.reshape(DM, HPC, P).transpose(1, 2, 0)).astype(bf),
        })
    return in_maps


def run(x, Wq, Wk, Wv, Wo, cfg=None, trace=False):
    cfg = cfg or CFG_FAST
    nc = _build(cfg)
    in_maps = _prep_inputs(x, Wq, Wk, Wv, Wo)
    try:
        res = run_bass_kernel_spmd(nc, in_maps, core_ids=list(range(NCORES)),
                                   trace=trace)
    except Exception:
        res = run_bass_kernel_spmd(nc, in_maps, core_ids=list(range(NCORES)),
                                   trace=trace)
    y = np.zeros((BT, DM), np.float32)
    for r in res.results:
        y += np.asarray(r["y"], np.float32)
    return y.reshape(B, T, DM), res


def kernel(x, Wq, Wk, Wv, Wo):
    y, _ = run(x, Wq, Wk, Wv, Wo)
    return y
